# revision 29
# baseline (speedup 1.0000x reference)
"""Trainium2 Bass kernel for ConcentrationLoss (fp8 streaming version).

Math (per batch element b, fully independent across b):
    g      = grid[b] viewed as (2, 4096)            # channels x pixels
    coord1 = g @ aff[b]                             # (2, 4096), the heavy op
    view coord1 as (2, 64, 64); extract 8x8 windows stride 4 -> 15x15 windows
    loss contribution = sum over windows w of [ sum_{p in w} x_p^2 - (sum_{p in w} x_p)^2 / 64 ]
    final = sum_b contribution_b / (8 * 2 * 225 * 64)

Sharding: batch b -> core b (8 cores).

The kernel is HBM-bandwidth bound: every byte of aff must be read once.
The loss tolerance (2e-2) is ~3500x above the baseline fp32 error, so aff
and g are quantized host-side to fp8 e4m3 (rel bias ~6e-3 on the loss,
measured against the fp32 oracle on CPU), cutting the stream from 64MB to
16MB per core. The matmul runs in DoubleRow fp8 mode.

Pipeline (trace-driven design; earlier revisions in git-less history):
  - The whole 16MB aff stream is SBUF-resident (128KB/partition of
    208KB), so every DMA dispatch is wait-free and the SDMA rings
    stream back-to-back at the HBM arbitration rate (~360-425 GB/s/core
    with all 8 cores streaming). A previous slot-recycled version gated
    DMA dispatch on matmul-count semaphores, which serialized the
    ~2.5-4us semaphore receipt latency into the stream and collapsed
    the back half to <300 GB/s.
  - Transfer 0 packs gt (the fp8 g chunk pairs, 64KB) together with g0
    pairs 0-3, so the weights and the first moving data arrive under
    one dispatch/semaphore; the LDWEIGHTS/matmul APs are hand-built
    views into that tile.
  - PE pre-warm: HAM boots the PE clock-gated at 1.2 GHz and releases
    it after ~3.4us of sustained activity; ~8 dead matmuls on a
    memset junk tile (issued before any data dependency) flip it by
    ~t=10.5us so the real stream runs at 2.4 GHz from the first tile.
    Two filler matmuls after each early group bridge the coarse early
    sem gaps so the gate never re-closes.
  - Matmuls: DoubleRow fp8, lhsT[128,2,2] x rhs[128,2,512] accumulating
    (2,512) PSUM banks; per-group windowed reduction (square on ACT,
    4-dim overlapping-AP tensor_reduce on DVE) overlaps the stream.
    Group 6 accumulates as two column halves in separate banks so its
    reduction overlaps group 7's matmuls.
  - Group 7 skips the on-device reduction: ACT copies move its raw
    coord1 rows to SBUF (PSUM has no DMA route) in three shrinking
    column blocks (256/128/128) and the host folds window rows 13-14.
    The final 0.25MB transfer's post-semaphore chain is just 8 FD-128
    matmuls + a 128-col copy + one outc DMA dispatch.
  - Host: loss_b = sumSSq - sumS2 / 64 per core, all-reduced on host.
"""

import numpy as np

B = 8
C = 2
H = W = 64
PIX = H * W  # 4096, contraction dim
WIN = 8
STRIDE = 4
OH = OW = 15
KC = PIX // 128   # 32 contraction chunks of 128
NPAIR = KC // 2   # 16 DoubleRow chunk pairs
NT = PIX // 512   # 8 column groups == psum banks
ROWS_PER_BANK = 512 // W  # 8 image rows per psum bank
GPAD = 16         # gt inner stride (pad 2 channels to 16B for DoubleRow LDW)
QP = NPAIR // 4   # pairs per g0 quarter-tile
HP = NPAIR // 2   # pairs per half-tile

_CACHE = {}


def _split_multi_waits(nc, limit=1):
    """The walrus build in this toolchain rejects instructions carrying more
    than one sync wait (any template: CTRL, S3_LW, ...). Tile's scheduler
    freely emits multi-wait instructions. Post-process the scheduled BIR:
    hoist excess waits onto one-wait NoOps inserted immediately before the
    instruction on the same engine (sequencer waits are conjunctive and
    blocking, so semantics are identical)."""
    import concourse.mybir as mybir

    n_split = 0
    for f in nc.m.functions:
        for b in f.blocks:
            insts = b.instructions  # live view
            i = 0
            while i < len(insts):
                inst = insts[i]
                si = inst.sync_info
                if si is not None and len(si.on_wait) > limit:
                    waits = list(si.on_wait)
                    extra, keep = waits[:-limit], waits[-limit:]
                    for w in extra:
                        nop = mybir.InstNoOp(name=f"SWS-{n_split}")
                        n_split += 1
                        nop.engine = inst.engine
                        nop.sync_info = mybir.SyncInfo(on_wait=[w], on_update=[])
                        insts.insert(i, nop)
                        i += 1
                    inst.sync_info = mybir.SyncInfo(
                        on_wait=keep, on_update=si.on_update
                    )
                i += 1
    return n_split


def _build_nc(split=True):
    import concourse.bass as bass
    import concourse.mybir as mybir
    import concourse.tile as tile

    f32 = mybir.dt.float32
    f8 = mybir.dt.float8e4
    nc = bass.Bass()
    # aff groups 0-6 pre-packed on the host: [group, p, pair, i, n]; element
    # (row, col) of aff[b] with row = 256*u + 128*i + p, col = 512*g + n
    # lands at [g, p, u, i, n].
    aff = nc.dram_tensor("aff", [NT - 1, 128, NPAIR, 2, 512], f8, kind="ExternalInput")
    # group 7 packed as three column blocks (256/128/128 cols) so the
    # PSUM->SBUF copies finish in shrinking chunks and the post-stream
    # chain after the very last (0.25MB) transfer is just 8 FD-128
    # matmuls + a 128-col copy + its DMA.
    aff7a = nc.dram_tensor("aff7a", [128, NPAIR, 2, 256], f8, kind="ExternalInput")
    aff7b = nc.dram_tensor("aff7b", [128, NPAIR, 2, 128], f8, kind="ExternalInput")
    aff7c = nc.dram_tensor("aff7c", [128, NPAIR, 2, 128], f8, kind="ExternalInput")
    # t0 = gt ++ g0 pairs 0-3, one transfer: per partition [gt 512B | pair0
    # 1KB | .. | pair3 1KB]. gt[p, kc, 0:2] = g[c, 128*kc + p], inner dim
    # padded to GPAD so the DoubleRow weight pair stride is 16B. Folding gt
    # into the first aff transfer removes a dispatch slot (~0.7us) from the
    # head of the sync queue and one semaphore from the first matmul.
    t0 = nc.dram_tensor("t0", [128, 512 + QP * 2 * 512], f8, kind="ExternalInput")
    # the last group's windowed reduction moves to the host: the device ships
    # the raw coord1 rows 56-63 plus the w-window sums for rows 52-55, and
    # the host folds window rows 13-14 into the final scalar.
    out = nc.dram_tensor("out", [C, 2], f32, kind="ExternalOutput")
    outc = nc.dram_tensor("outc", [C, 512], f32, kind="ExternalOutput")
    outy = nc.dram_tensor("outy", [C, 4 * OW], f32, kind="ExternalOutput")
    outq = nc.dram_tensor("outq", [C, 4 * OW], f32, kind="ExternalOutput")

    with tile.TileContext(nc) as tc:
        with (
            tc.tile_pool(name="consts", bufs=1) as consts,
            tc.tile_pool(name="small", bufs=1) as small,
            tc.tile_pool(name="sqp", bufs=4) as sqp,
            tc.tile_pool(name="affr", bufs=1) as affr,
            tc.tile_pool(name="ps1", bufs=8, space="PSUM") as ps1,
        ):
            # t0 (gt + g0 quarter 0) first on the HWDGE sync queue
            t0_sb = consts.tile([128, 512 + QP * 2 * 512], f8)
            nc.sync.dma_start(out=t0_sb, in_=t0[:, :])
            t0b = t0_sb[:, :]

            def ldw(u):
                """weight-pair AP for contraction pair u inside t0's gt
                region: [part][kc pair, stride GPAD][channel]"""
                return bass.AP(
                    tensor=t0b.tensor,
                    offset=t0b.offset + 2 * GPAD * u,
                    ap=[list(t0b.ap[0]), [GPAD, 2], [1, 2]],
                )

            def rhs0(u):
                """moving-operand AP for g0 pair u<4 inside t0: [part][i][n]"""
                return bass.AP(
                    tensor=t0b.tensor,
                    offset=t0b.offset + 512 + 1024 * u,
                    ap=[list(t0b.ap[0]), [512, 2], [1, 512]],
                )

            # PE pre-warm: HAM boots the PE clock-gated to 1.2 GHz and only
            # releases after ~3.4us of sustained activity. Memset a junk
            # tile (vector engine, idle at this point) and burn ~10 dead
            # matmuls on it before the first data lands, so the real matmul
            # stream starts at 2.4 GHz.
            junk = consts.tile([128, KC, GPAD], f8)
            nc.vector.memset(junk, 0.0)

            # the whole aff stream is resident: one SBUF tile per group,
            # written by wait-free quarter/half transfers
            at = [
                affr.tile([128, NPAIR, 2, 512], f8, name=f"at{g}")
                for g in range(NT - 1)
            ]
            at7a = affr.tile([128, NPAIR, 2, 256], f8, name="at7a")
            at7b = affr.tile([128, NPAIR, 2, 128], f8, name="at7b")
            at7c = affr.tile([128, NPAIR, 2, 128], f8, name="at7c")
            # g0: quarter 0 rides in t0; quarters 1-3 transfer separately;
            # g1-g6: 2 half transfers each; g7: column blocks, the last
            # split in two by contraction pairs. All wait-free on sync.
            for q in range(1, 4):
                nc.sync.dma_start(
                    out=at[0][:, q * QP:(q + 1) * QP], in_=aff[0, :, q * QP:(q + 1) * QP]
                )
            # NOTE: keep transfers at <=1MB. Measured: a transfer's queue-
            # semaphore becomes consumer-visible ~(1us + one transfer-time)
            # after its last byte, so 2MB tiles release their matmul bursts
            # ~2.5us later than 1MB halves and the PE backlog propagates to
            # the kernel end (+3us end-to-end when g1/g2 were merged).
            # g3-g5 stream as 0.5MB quarters: smaller sem lag, and the +6
            # transfers (with the +1 g7c split below) rotate the final outc
            # DMA onto queue-sem lane 7 (S165) -- the LAST id in the
            # epilogue's ascending sem audit, so the whole audit chain
            # overlaps outc's write receipt instead of following it.
            for g in range(1, NT - 1):
                if 3 <= g <= 5:
                    for q in range(4):
                        nc.sync.dma_start(
                            out=at[g][:, q * QP:(q + 1) * QP],
                            in_=aff[g, :, q * QP:(q + 1) * QP],
                        )
                else:
                    for h in range(2):
                        nc.sync.dma_start(
                            out=at[g][:, h * HP:(h + 1) * HP],
                            in_=aff[g, :, h * HP:(h + 1) * HP],
                        )
            nc.sync.dma_start(out=at7a, in_=aff7a[:, :, :, :])
            nc.sync.dma_start(out=at7b, in_=aff7b[:, :, :, :])
            nc.sync.dma_start(out=at7c[:, 0:HP], in_=aff7c[:, 0:HP])
            nc.sync.dma_start(out=at7c[:, HP:HP + QP], in_=aff7c[:, HP:HP + QP])
            # final two 64KB pieces: the very last transfer's sem lag is
            # ~(1us + 0.15us) and its post-sem burst is just 2 matmuls
            nc.sync.dma_start(
                out=at7c[:, HP + QP:HP + QP + 2], in_=aff7c[:, HP + QP:HP + QP + 2]
            )
            nc.sync.dma_start(
                out=at7c[:, HP + QP + 2:NPAIR], in_=aff7c[:, HP + QP + 2:NPAIR]
            )

            y_sb = small.tile([C, H, OW], f32)      # w-windowsums of x
            ysq_sb = small.tile([C, H, OW], f32)    # w-windowsums of x^2
            s_sb = small.tile([C, OH * OW], f32)    # full window sums
            ssq_sb = small.tile([C, OH * OW], f32)  # full window sums of x^2
            s2_sb = small.tile([C, OH * OW], f32)   # S^2
            # [ssq{rows 0-12}, s2{0-12}]: window rows 13-14 finish on the host
            out_sb = small.tile([C, 2], f32)
            cp_sb = small.tile([C, 512], f32)       # raw coord1 rows 56-63

            def windowed(ap, row_step, n_rows):
                """4-dim overlapping AP: [part, row, window j, dw] over a
                (C, n_rows*row_step) region; one tensor_reduce(X) gives the
                w-direction window sums in a single instruction."""
                return bass.AP(
                    tensor=ap.tensor,
                    offset=ap.offset,
                    ap=[list(ap.ap[0]), [row_step, n_rows], [STRIDE, OW], [1, WIN]],
                )

            def bank_postprocess(r0, nrows, bank):
                """w-direction window sums for image rows [r0, r0+nrows);
                bank is the (C, nrows*W) psum region holding them."""
                sq = sqp.tile([C, 512], f32, tag="sq")
                nc.scalar.square(out=sq[:, :nrows * W], in_=bank)
                yd = y_sb[:, r0:r0 + nrows, :]
                qd = ysq_sb[:, r0:r0 + nrows, :]
                nc.vector.reduce_sum(
                    out=yd, in_=windowed(bank, W, nrows),
                    axis=mybir.AxisListType.X,
                )
                nc.vector.reduce_sum(
                    out=qd, in_=windowed(sq[:, :nrows * W], W, nrows),
                    axis=mybir.AxisListType.X,
                )

            # h-direction window sums, incremental: S[c, i, j] = sum_dh
            # Y[c, 4i+dh, j]. Window row i needs Y rows 4i..4i+7; after bank
            # n the rows up to 8n+7 exist, so rows {2n-1, 2n} (and row 0 for
            # n=0) become computable.
            sv = s_sb.rearrange("c (i j) -> c i j", j=OW)
            qv = ssq_sb.rearrange("c (i j) -> c i j", j=OW)

            def h_rows(i0, cnt):
                for src, dst in ((y_sb, sv), (ysq_sb, qv)):
                    ap = src[:, :, :]
                    win = bass.AP(
                        tensor=ap.tensor,
                        offset=ap.offset + i0 * STRIDE * OW,
                        ap=[list(ap.ap[0]), [STRIDE * OW, cnt], [1, OW], [OW, WIN]],
                    )
                    nc.vector.reduce_sum(
                        out=dst[:, i0:i0 + cnt, :], in_=win,
                        axis=mybir.AxisListType.X,
                    )

            def final_rows(i0, n, slot):
                """fold finished S/SSq window rows [i0, i0+n) into out_sb."""
                a, b = i0 * OW, (i0 + n) * OW
                nc.scalar.square(out=s2_sb[:, a:b], in_=s_sb[:, a:b])
                nc.vector.reduce_sum(
                    out=out_sb[:, slot:slot + 1], in_=ssq_sb[:, a:b],
                    axis=mybir.AxisListType.X,
                )
                nc.vector.reduce_sum(
                    out=out_sb[:, slot + 1:slot + 2], in_=s2_sb[:, a:b],
                    axis=mybir.AxisListType.X,
                )

            # PSUM: groups 0-5 get full (2,512) banks; group 6 two column
            # halves (postprocess of the left overlaps matmuls of the
            # right); group 7 two column halves (copy of the left overlaps
            # matmuls of the right). 10 allocations rotate 8 banks; the
            # reused slots' readers finish ~30us before the rewrite.
            c1bs = [
                ps1.tile([C, 512], f32, tag="bank", bufs=7, name=f"c1b{g}")
                for g in range(NT - 2)
            ]
            chalf = {
                s: ps1.tile([C, 256], f32, tag="bank", bufs=7, name=f"c1{s}6")
                for s in ("l", "r")
            }
            c7a = ps1.tile([C, 256], f32, tag="bank", bufs=7, name="c1g7a")
            c7b = ps1.tile([C, 128], f32, tag="bank", bufs=7, name="c1g7b")
            c7c = ps1.tile([C, 128], f32, tag="bank", bufs=7, name="c1g7c")
            c1w = ps1.tile([C, 512], f32, tag="warm", bufs=1, name="c1w")

            def warm(n):
                """Dead matmuls on the junk tile: keep the HAM clock gate
                open. Waitless, so they run in PE idle gaps and are
                absorbed by the PE's slack vs the stream."""
                for _ in range(n):
                    nc.tensor.matmul(
                        c1w,
                        lhsT=junk[:, 0:1, 0:2],
                        rhs=junk[:, :, :],
                        start=True,
                        stop=True,
                    )

            # release the HAM clock gate before the first aff tile's
            # semaphore fires (~8 cold matmuls span ~4.5us from ~t=8us)
            warm(8)

            # group 0: quarter 0 streams from t0, quarters 1-3 from at[0]
            for q in range(4):
                for u in range(q * QP, (q + 1) * QP):
                    nc.tensor.matmul(
                        c1bs[0],
                        lhsT=ldw(u),
                        rhs=rhs0(u) if u < QP else at[0][:, u, :, :],
                        start=(u == 0),
                        stop=(u == NPAIR - 1),
                        perf_mode=mybir.MatmulPerfMode.DoubleRow,
                    )
            bank_postprocess(0, 8, c1bs[0])
            h_rows(0, 1)
            warm(2)

            # groups 1-5: halves
            for g in range(1, NT - 2):
                for u in range(NPAIR):
                    nc.tensor.matmul(
                        c1bs[g],
                        lhsT=ldw(u),
                        rhs=at[g][:, u, :, :],
                        start=(u == 0),
                        stop=(u == NPAIR - 1),
                        perf_mode=mybir.MatmulPerfMode.DoubleRow,
                    )
                bank_postprocess(8 * g, 8, c1bs[g])
                h_rows(2 * g - 1, 2)
                if g <= 2:
                    # early phase: stream pacing still coarse; keep the HAM
                    # activity window fed through the sem-wait gaps
                    warm(2)

            # group 6: column-split accumulation; left finishes first
            g = NT - 2
            for bank, n0, n1 in ((chalf["l"], 0, 256), (chalf["r"], 256, 512)):
                for u in range(HP):
                    nc.tensor.matmul(
                        bank,
                        lhsT=ldw(u),
                        rhs=at[g][:, u, :, n0:n1],
                        start=(u == 0),
                        stop=False,
                        perf_mode=mybir.MatmulPerfMode.DoubleRow,
                    )
            for u in range(HP, NPAIR):
                nc.tensor.matmul(
                    chalf["l"],
                    lhsT=ldw(u),
                    rhs=at[g][:, u, :, 0:256],
                    start=False,
                    stop=(u == NPAIR - 1),
                    perf_mode=mybir.MatmulPerfMode.DoubleRow,
                )
            bank_postprocess(48, 4, chalf["l"])
            h_rows(11, 1)
            for u in range(HP, NPAIR):
                nc.tensor.matmul(
                    chalf["r"],
                    lhsT=ldw(u),
                    rhs=at[g][:, u, :, 256:512],
                    start=False,
                    stop=(u == NPAIR - 1),
                    perf_mode=mybir.MatmulPerfMode.DoubleRow,
                )
            bank_postprocess(52, 4, chalf["r"])
            h_rows(12, 1)
            # rows 0-12 of S/SSq are final: fold them down now
            final_rows(0, 13, 0)

            # group 7: three column blocks, each full contraction; copy each
            # off PSUM as it completes (host does the windowed reduction).
            # Copies run on the scalar engine; the outc DMAs dispatch from
            # sync so copy k+1 never queues behind DMA k's dispatch.
            for bank, src, n0, n1 in (
                (c7a, at7a, 0, 256),
                (c7b, at7b, 256, 384),
                (c7c, at7c, 384, 512),
            ):
                for u in range(NPAIR):
                    nc.tensor.matmul(
                        bank,
                        lhsT=ldw(u),
                        rhs=src[:, u, :, :],
                        start=(u == 0),
                        stop=(u == NPAIR - 1),
                        perf_mode=mybir.MatmulPerfMode.DoubleRow,
                    )
                nc.scalar.copy(out=cp_sb[:, n0:n1], in_=bank)

            nc.sync.dma_start(out=outy[:, :], in_=y_sb[:, 52:56, :])
            nc.sync.dma_start(out=outq[:, :], in_=ysq_sb[:, 52:56, :])
            nc.sync.dma_start(out=out[:, :], in_=out_sb)
            # single outc DMA: three serialized ~0.7us dispatches on sync
            # were the tail's critical path; one dispatch after the last
            # copy is cheaper than overlapping the first two copies' data
            nc.sync.dma_start(out=outc[:, :], in_=cp_sb)
    if split:
        _split_multi_waits(nc)
    return nc


def _f8(x):
    import ml_dtypes

    return np.asarray(x, dtype=np.float32).astype(ml_dtypes.float8_e4m3)


def _gt_host(grid_b):
    # grid_b: (64, 64, 2). g[c, p] = grid_b.reshape(4096, 2)[p, c]
    # gt layout: gt[p, kc, c] = g[c, 128*kc + p], inner padded to GPAD
    import ml_dtypes

    g = np.ascontiguousarray(grid_b, dtype=np.float32).reshape(PIX, C)
    gt = np.zeros((128, KC, GPAD), dtype=ml_dtypes.float8_e4m3)
    gt[:, :, :C] = _f8(g.reshape(KC, 128, C).transpose(1, 0, 2))
    return gt


def _aff_host(aff_b):
    # pack into DMA-tile order [g, p, u, i, n]:
    # element (row, col) with row = 256u + 128i + p, col = 512g + n.
    # group 7 additionally splits into two contiguous column halves.
    a8 = _f8(aff_b)  # (4096, 4096)
    a8 = a8.reshape(NPAIR, 2, 128, NT, 512).transpose(3, 2, 0, 1, 4)
    main = np.ascontiguousarray(a8[: NT - 1])
    g7 = a8[NT - 1]  # (128, NPAIR, 2, 512)
    return (
        main,
        np.ascontiguousarray(g7[..., 0:256]),
        np.ascontiguousarray(g7[..., 256:384]),
        np.ascontiguousarray(g7[..., 384:512]),
    )


def run_cores(aff, grid, trace=False):
    """Compile (cached) and run the per-core bass kernel on cores 0..7.

    Returns the BassKernelResults from run_bass_kernel_spmd."""
    from concourse.bass_utils import run_bass_kernel_spmd

    if "nc" not in _CACHE:
        _CACHE["nc"] = _build_nc()
    nc = _CACHE["nc"]

    in_maps = []
    for b in range(B):
        main, a7a, a7b, a7c = _aff_host(aff[b])
        # t0 = [gt | g0 pairs 0-3] per partition
        gt = _gt_host(grid[b]).reshape(128, 512)
        q0 = main[0, :, 0:QP].reshape(128, QP * 1024)
        t0 = np.ascontiguousarray(np.concatenate([gt, q0], axis=1))
        in_maps.append(
            {"aff": main, "aff7a": a7a, "aff7b": a7b, "aff7c": a7c, "t0": t0}
        )
    return run_bass_kernel_spmd(nc, in_maps, core_ids=list(range(B)), trace=trace)


def kernel(aff, grid):
    aff = np.asarray(aff, dtype=np.float32)
    grid = np.asarray(grid, dtype=np.float32)
    res = run_cores(aff, grid)
    total = 0.0
    for b in range(B):
        r = res.results[b]
        o = r["out"].astype(np.float64)
        # device ships raw coord1 rows 56-63 + w-window sums for rows 52-55;
        # fold window rows 13-14 here
        x = r["outc"].astype(np.float64).reshape(C, 8, W)
        idx = (STRIDE * np.arange(OW))[:, None] + np.arange(WIN)[None, :]
        yh = x[:, :, idx].sum(axis=3)          # (C, 8, OW) w-sums of x
        qh = (x ** 2)[:, :, idx].sum(axis=3)   # (C, 8, OW) w-sums of x^2
        y52 = r["outy"].astype(np.float64).reshape(C, 4, OW)
        q52 = r["outq"].astype(np.float64).reshape(C, 4, OW)
        s13 = y52.sum(axis=1) + yh[:, :4].sum(axis=1)   # rows 52-59
        ssq13 = q52.sum(axis=1) + qh[:, :4].sum(axis=1)
        s14 = yh.sum(axis=1)                            # rows 56-63
        ssq14 = qh.sum(axis=1)
        ssq = o[:, 0].sum() + ssq13.sum() + ssq14.sum()
        s2 = o[:, 1].sum() + (s13 ** 2).sum() + (s14 ** 2).sum()
        total += ssq - s2 / (WIN * WIN)
    total /= B * C * OH * OW * WIN * WIN
    return np.asarray(total, dtype=np.float32)
